# revision 2
# baseline (speedup 1.0000x reference)
"""Trainium2 Bass kernel for nn_HLALayer (higher-order linear attention).

Math: the reference scan
    k_C = k_t @ C;  G += k_t k_t^T C;  S += k_t k_t^T;  C += q_t v_t^T
    o_t = q_t @ (S C - G)
admits the closed form o_t = sum_{r<=t} (q_t^T S(r) q_r) v_r, which chunks
(chunk L) into:
    o_chunk = Q @ D0 + tril(Q S0 Q^T + M tril(M)^T) @ V,   M = Q K^T
with per-chunk state updates
    dS = K^T K;  dC = Q^T V;  G1 = G0 + dS C0 + K^T(stril(K Q^T) V)
    S1 = S0+dS; C1 = C0+dC; D1 = S1 C1 - G1
All matmuls use float32r (TF32-like, 1 cyc/row at free-dim>=256).

Sharding: 8 cores = (batch b in {0,1}) x (head-group g in {0..3}, 4 heads
each).  Each core projects x[b] with its weight column/row slices, runs the
chunked scan for its 4 heads, applies its W_o row-slice -> partial [T, D]
output; the host sums the 4 partials per batch.
"""

import numpy as np
import sys

sys.path.insert(0, "/opt/trn_rl_repo")

import concourse.bacc as bacc
import concourse.mybir as mybir
from concourse.bass_utils import run_bass_kernel_spmd
from concourse.tile import TileContext

F32 = mybir.dt.float32
F32R = mybir.dt.float32r

D = 1024          # model dim
DL = 256          # per-core projection width (4 heads x 64)
DK = 64           # head dim
L = 256           # chunk length
NHL = 4           # local heads per core


def build(T=2048):
    NCH = T // L
    nc = bacc.Bacc("TRN2", target_bir_lowering=False)

    x_in = nc.declare_dram_parameter("x", [T, D], F32R, isOutput=False)
    wq_in = nc.declare_dram_parameter("wq", [D, DL], F32R, isOutput=False)
    wk_in = nc.declare_dram_parameter("wk", [D, DL], F32R, isOutput=False)
    wv_in = nc.declare_dram_parameter("wv", [D, DL], F32R, isOutput=False)
    wo_in = nc.declare_dram_parameter("wo", [DL, D], F32R, isOutput=False)
    id_in = nc.declare_dram_parameter("ident", [128, 128], F32R, isOutput=False)
    # masks (f = free index, p = partition index)
    mt0_in = nc.declare_dram_parameter("mt0", [128, 256], F32, isOutput=False)  # [triu|1]
    ms0_in = nc.declare_dram_parameter("ms0", [128, 256], F32, isOutput=False)  # [striu|1]
    mz1_in = nc.declare_dram_parameter("mz1", [128, 256], F32, isOutput=False)  # [0|triu]
    mtr_in = nc.declare_dram_parameter("mtr", [128, 128], F32, isOutput=False)  # triu
    mst_in = nc.declare_dram_parameter("mst", [128, 128], F32, isOutput=False)  # striu
    out_d = nc.declare_dram_parameter("out", [T, D], F32, isOutput=True)

    ncp = 0  # copy-engine round robin counter

    with TileContext(nc) as tc:
        with tc.tile_pool(name="const", bufs=1) as cpool, \
             tc.tile_pool(name="work", bufs=2) as work, \
             tc.tile_pool(name="spool", bufs=2) as spool, \
             tc.tile_pool(name="pp", bufs=2, space="PSUM") as pps:

            def cp(out_ap, in_ap):
                """plain copy, alternating DVE / ACT to balance load"""
                nonlocal ncp
                ncp += 1
                if ncp % 2 == 0:
                    nc.vector.tensor_copy(out_ap, in_ap)
                else:
                    nc.scalar.copy(out_ap, in_ap)

            # ---- constants / weights (gpsimd=SWDGE queue, keeps HWDGE free) ----
            ident = cpool.tile([128, 128], F32R)
            nc.gpsimd.dma_start(out=ident[:], in_=id_in[:])
            mt0 = cpool.tile([128, 256], F32)
            nc.gpsimd.dma_start(out=mt0[:], in_=mt0_in[:])
            ms0 = cpool.tile([128, 256], F32)
            nc.gpsimd.dma_start(out=ms0[:], in_=ms0_in[:])
            mz1 = cpool.tile([128, 256], F32)
            nc.gpsimd.dma_start(out=mz1[:], in_=mz1_in[:])
            mtr = cpool.tile([128, 128], F32)
            nc.gpsimd.dma_start(out=mtr[:], in_=mtr_in[:])
            mst = cpool.tile([128, 128], F32)
            nc.gpsimd.dma_start(out=mst[:], in_=mst_in[:])

            wq_sb, wk_sb, wv_sb = [], [], []
            for j in range(8):
                wqt = cpool.tile([128, DL], F32R, name=f"wq{j}")
                nc.gpsimd.dma_start(out=wqt[:], in_=wq_in[128 * j:128 * (j + 1), :])
                wq_sb.append(wqt)
                wkt = cpool.tile([128, DL], F32R, name=f"wk{j}")
                nc.gpsimd.dma_start(out=wkt[:], in_=wk_in[128 * j:128 * (j + 1), :])
                wk_sb.append(wkt)
                wvt = cpool.tile([128, DL], F32R, name=f"wv{j}")
                nc.gpsimd.dma_start(out=wvt[:], in_=wv_in[128 * j:128 * (j + 1), :])
                wv_sb.append(wvt)
            wo_sb = []
            for m in range(2):
                wot = cpool.tile([128, D], F32R, name=f"wo{m}")
                nc.gpsimd.dma_start(out=wot[:], in_=wo_in[128 * m:128 * (m + 1), :])
                wo_sb.append(wot)

            # per-head states
            S = [None] * NHL
            C = [None] * NHL
            G = [None] * NHL
            Dst = [None] * NHL

            for c in range(NCH):
                t0 = L * c
                # ---------- phase 1: x^T tiles via PE transpose ----------
                xn = []
                for bb in range(2):
                    xnb = work.tile([128, D], F32R, tag=f"xn{bb}", bufs=2,
                                    name=f"xn{bb}_{c}")
                    nc.sync.dma_start(out=xnb[:],
                                      in_=x_in[t0 + 128 * bb:t0 + 128 * (bb + 1), :])
                    xn.append(xnb)
                xt = []
                for j in range(8):
                    xtj = work.tile([128, L], F32R, tag=f"xt{j}", bufs=2,
                                    name=f"xt{j}_{c}")
                    for bb in range(2):
                        pstx = pps.tile([128, 128], F32R, tag="ps", bufs=3,
                                        name=f"pstx{j}{bb}_{c}")
                        nc.tensor.transpose(pstx[:], xn[bb][:, 128 * j:128 * (j + 1)],
                                            ident[:])
                        cp(xtj[:, 128 * bb:128 * (bb + 1)], pstx[:])
                    xt.append(xtj)

                # projections: QT/KT [dk-tile 128, t 256], V natural [t 128, dv 256]
                qt, kt = [], []
                for m in range(2):
                    psq = pps.tile([128, L], F32, tag="pb", bufs=5, name=f"psq{m}_{c}")
                    for j in range(8):
                        nc.tensor.matmul(psq[:], wq_sb[j][:, 128 * m:128 * (m + 1)],
                                         xt[j][:], start=(j == 0), stop=(j == 7))
                    qtm = work.tile([128, L], F32R, tag=f"qt{m}", bufs=2,
                                    name=f"qt{m}_{c}")
                    cp(qtm[:], psq[:])
                    qt.append(qtm)
                    psk = pps.tile([128, L], F32, tag="pb", bufs=5, name=f"psk{m}_{c}")
                    for j in range(8):
                        nc.tensor.matmul(psk[:], wk_sb[j][:, 128 * m:128 * (m + 1)],
                                         xt[j][:], start=(j == 0), stop=(j == 7))
                    ktm = work.tile([128, L], F32R, tag=f"kt{m}", bufs=2,
                                    name=f"kt{m}_{c}")
                    cp(ktm[:], psk[:])
                    kt.append(ktm)
                vt = []
                for bb in range(2):
                    psv = pps.tile([128, DL], F32, tag="pb", bufs=5, name=f"psv{bb}_{c}")
                    for j in range(8):
                        nc.tensor.matmul(psv[:], xt[j][:, 128 * bb:128 * (bb + 1)],
                                         wv_sb[j][:], start=(j == 0), stop=(j == 7))
                    vtb = work.tile([128, DL], F32R, tag=f"vt{bb}", bufs=2,
                                    name=f"vt{bb}_{c}")
                    cp(vtb[:], psv[:])
                    vt.append(vtb)

                # per-chunk output tiles (oT layout [dv 256 -> 2 tiles, t 256])
                ot = [work.tile([128, L], F32R, tag=f"ot{m}", bufs=2,
                                name=f"ot{m}_{c}") for m in range(2)]

                # ---------- phase 2: chunked scan per local head ----------
                for h in range(NHL):
                    m = h >> 1
                    po = 64 * (h & 1)
                    QTc = qt[m][po:po + 64, :]
                    KTc = kt[m][po:po + 64, :]
                    Vb = [vt[bb][:, 64 * h:64 * h + 64] for bb in range(2)]
                    idb = ident[po:po + 64, po:po + 64]

                    # N = K Q^T [s, t];  M = Q K^T [t, s]
                    psN0 = pps.tile([128, L], F32, tag="pb", bufs=5, name=f"psN0_{c}_{h}")
                    nc.tensor.matmul(psN0[:], KTc[:, 0:128], QTc, start=True, stop=True)
                    psN1 = pps.tile([128, L], F32, tag="pb", bufs=5, name=f"psN1_{c}_{h}")
                    nc.tensor.matmul(psN1[:], KTc[:, 128:256], QTc, start=True, stop=True)
                    psM0 = pps.tile([128, L], F32, tag="pb", bufs=5, name=f"psM0_{c}_{h}")
                    nc.tensor.matmul(psM0[:], QTc[:, 0:128], KTc, start=True, stop=True)
                    psM1 = pps.tile([128, L], F32, tag="pb", bufs=5, name=f"psM1_{c}_{h}")
                    nc.tensor.matmul(psM1[:], QTc[:, 128:256], KTc, start=True, stop=True)

                    triuN0 = work.tile([128, L], F32R, tag="triuN0", bufs=2,
                                       name=f"triuN0_{c}_{h}")
                    nc.vector.tensor_mul(triuN0[:], psN0[:], mt0[:])
                    n0sb = work.tile([128, L], F32R, tag="n0sb", bufs=2,
                                     name=f"n0sb_{c}_{h}")
                    cp(n0sb[:], psN0[:])
                    triuN1 = work.tile([128, 128], F32R, tag="triuN1", bufs=2,
                                       name=f"triuN1_{c}_{h}")
                    nc.vector.tensor_mul(triuN1[:], psN1[:, 128:256], mtr[:])
                    n1sb = work.tile([128, L], F32R, tag="n1sb", bufs=2,
                                     name=f"n1sb_{c}_{h}")
                    cp(n1sb[:], psN1[:])
                    smM0 = work.tile([128, L], F32R, tag="smM0", bufs=2,
                                     name=f"smM0_{c}_{h}")
                    nc.vector.tensor_mul(smM0[:], psM0[:], ms0[:])
                    smM1 = work.tile([128, 128], F32R, tag="smM1", bufs=2,
                                     name=f"smM1_{c}_{h}")
                    nc.vector.tensor_mul(smM1[:], psM1[:, 128:256], mst[:])

                    # QST = S0 @ QTc  [dk, t]
                    if c > 0:
                        psQST = pps.tile([64, L], F32, tag="ps", bufs=3,
                                         name=f"psQST_{c}_{h}")
                        nc.tensor.matmul(psQST[:], S[h][po:po + 64, :], QTc,
                                         start=True, stop=True)
                        qstsb = work.tile([128, L], F32R, tag="qst", bufs=2,
                                          name=f"qst_{c}_{h}")
                        cp(qstsb[po:po + 64, :], psQST[:])

                    # AT = PT + AqsT  [r, t]
                    psAT0 = pps.tile([128, L], F32, tag="pb", bufs=5,
                                     name=f"psAT0_{c}_{h}")
                    nc.tensor.matmul(psAT0[:], triuN0[:, 0:128], n0sb[:],
                                     start=True, stop=(c == 0))
                    if c > 0:
                        nc.tensor.matmul(psAT0[:], qstsb[po:po + 64, 0:128], QTc,
                                         start=False, stop=True)
                    psAT1 = pps.tile([128, L], F32, tag="pb", bufs=5,
                                     name=f"psAT1_{c}_{h}")
                    nc.tensor.matmul(psAT1[:], triuN0[:, 128:256], n0sb[:],
                                     start=True, stop=False)
                    nc.tensor.matmul(psAT1[:], triuN1[:], n1sb[:],
                                     start=False, stop=(c == 0))
                    if c > 0:
                        nc.tensor.matmul(psAT1[:], qstsb[po:po + 64, 128:256], QTc,
                                         start=False, stop=True)
                    at0 = work.tile([128, L], F32R, tag="at0", bufs=2,
                                    name=f"at0_{c}_{h}")
                    nc.vector.tensor_mul(at0[:], psAT0[:], mt0[:])
                    at1 = work.tile([128, L], F32R, tag="at1", bufs=2,
                                    name=f"at1_{c}_{h}")
                    nc.vector.tensor_mul(at1[:], psAT1[:], mz1[:])

                    # oT = V^T AT + (Q D0)^T   [dv, t]
                    psO = pps.tile([64, L], F32, tag="ps", bufs=3, name=f"psO_{c}_{h}")
                    nc.tensor.matmul(psO[:], Vb[0], at0[:], start=True, stop=False)
                    nc.tensor.matmul(psO[:], Vb[1], at1[:], start=False, stop=(c == 0))
                    if c > 0:
                        nc.tensor.matmul(psO[:], Dst[h][po:po + 64, :], QTc, start=False, stop=True)
                    cp(ot[m][po:po + 64, :], psO[:])

                    # natural-layout Q, K via PE transpose
                    qn, kn = [], []
                    for bb in range(2):
                        psq_t = pps.tile([128, 64], F32R, tag="ps", bufs=3,
                                         name=f"psqn{bb}_{c}_{h}")
                        nc.tensor.transpose(psq_t[:], QTc[:, 128 * bb:128 * (bb + 1)], idb)
                        qnb = work.tile([128, 64], F32R, tag=f"qn{bb}", bufs=2,
                                        name=f"qn{bb}_{c}_{h}")
                        cp(qnb[:], psq_t[:])
                        qn.append(qnb)
                        psk_t = pps.tile([128, 64], F32R, tag="ps", bufs=3,
                                         name=f"pskn{bb}_{c}_{h}")
                        nc.tensor.transpose(psk_t[:], KTc[:, 128 * bb:128 * (bb + 1)], idb)
                        knb = work.tile([128, 64], F32R, tag=f"kn{bb}", bufs=2,
                                        name=f"kn{bb}_{c}_{h}")
                        cp(knb[:], psk_t[:])
                        kn.append(knb)

                    # dS = K^T K, dC = Q^T V
                    psS = pps.tile([64, 64], F32, tag="ps", bufs=3, name=f"psS_{c}_{h}")
                    nc.tensor.matmul(psS[:], kn[0][:], kn[0][:], start=True, stop=False)
                    nc.tensor.matmul(psS[:], kn[1][:], kn[1][:], start=False, stop=True)
                    psC = pps.tile([64, 64], F32, tag="ps", bufs=3, name=f"psC_{c}_{h}")
                    nc.tensor.matmul(psC[:], qn[0][:], Vb[0], start=True, stop=False)
                    nc.tensor.matmul(psC[:], qn[1][:], Vb[1], start=False, stop=True)

                    # W2 = stril(N) V  [s, dv]
                    psW0 = pps.tile([128, 64], F32, tag="ps", bufs=3,
                                    name=f"psW0_{c}_{h}")
                    nc.tensor.matmul(psW0[:], smM0[:, 0:128], Vb[0], start=True, stop=True)
                    psW1 = pps.tile([128, 64], F32, tag="ps", bufs=3,
                                    name=f"psW1_{c}_{h}")
                    nc.tensor.matmul(psW1[:], smM0[:, 128:256], Vb[0], start=True, stop=False)
                    nc.tensor.matmul(psW1[:], smM1[:], Vb[1], start=False, stop=True)
                    w0sb = work.tile([128, 64], F32R, tag="w0sb", bufs=2,
                                     name=f"w0sb_{c}_{h}")
                    cp(w0sb[:], psW0[:])
                    w1sb = work.tile([128, 64], F32R, tag="w1sb", bufs=2,
                                     name=f"w1sb_{c}_{h}")
                    cp(w1sb[:], psW1[:])

                    # G update: Gamma = K^T W2 (+ dS C0)
                    psG = pps.tile([64, 64], F32, tag="ps", bufs=3, name=f"psG_{c}_{h}")
                    nc.tensor.matmul(psG[:], kn[0][:], w0sb[:], start=True, stop=False)
                    nc.tensor.matmul(psG[:], kn[1][:], w1sb[:], start=False, stop=(c == 0))
                    if c > 0:
                        dssb = work.tile([128, 64], F32R, tag="dssb", bufs=2,
                                         name=f"dssb_{c}_{h}")
                        cp(dssb[po:po + 64, :], psS[:])
                        nc.tensor.matmul(psG[:], dssb[po:po + 64, :], C[h][po:po + 64, :],
                                         start=False, stop=True)

                    # new states
                    Snew = spool.tile([128, 64], F32R, tag=f"S{h}", bufs=2,
                                      name=f"S{h}_{c}")
                    Cnew = spool.tile([128, 64], F32R, tag=f"C{h}", bufs=2,
                                      name=f"C{h}_{c}")
                    Gnew = spool.tile([128, 64], F32R, tag=f"G{h}", bufs=2,
                                      name=f"G{h}_{c}")
                    if c > 0:
                        nc.vector.tensor_add(Snew[po:po + 64, :], S[h][po:po + 64, :], psS[:])
                        nc.vector.tensor_add(Cnew[po:po + 64, :], C[h][po:po + 64, :], psC[:])
                        nc.vector.tensor_add(Gnew[po:po + 64, :], G[h][po:po + 64, :], psG[:])
                    else:
                        nc.vector.tensor_copy(Snew[po:po + 64, :], psS[:])
                        nc.vector.tensor_copy(Cnew[po:po + 64, :], psC[:])
                        nc.vector.tensor_copy(Gnew[po:po + 64, :], psG[:])
                    psD = pps.tile([64, 64], F32, tag="ps", bufs=3, name=f"psD_{c}_{h}")
                    nc.tensor.matmul(psD[:], Snew[po:po + 64, :], Cnew[po:po + 64, :],
                                     start=True, stop=True)
                    Dnew = spool.tile([128, 64], F32R, tag=f"D{h}", bufs=2,
                                      name=f"D{h}_{c}")
                    nc.vector.tensor_sub(Dnew[po:po + 64, :], psD[:], Gnew[po:po + 64, :])
                    S[h], C[h], G[h], Dst[h] = Snew, Cnew, Gnew, Dnew

                # ---------- phase 3: output projection for this chunk ----------
                for bb in range(2):
                    for ncol in range(2):
                        pso = pps.tile([128, 512], F32, tag="pb", bufs=5,
                                       name=f"pso{bb}{ncol}_{c}")
                        nc.tensor.matmul(pso[:], ot[0][:, 128 * bb:128 * (bb + 1)],
                                         wo_sb[0][:, 512 * ncol:512 * (ncol + 1)],
                                         start=True, stop=False)
                        nc.tensor.matmul(pso[:], ot[1][:, 128 * bb:128 * (bb + 1)],
                                         wo_sb[1][:, 512 * ncol:512 * (ncol + 1)],
                                         start=False, stop=True)
                        osb = work.tile([128, 512], F32, tag="osb", bufs=3,
                                        name=f"osb{bb}{ncol}_{c}")
                        cp(osb[:], pso[:])
                        nc.scalar.dma_start(
                            out=out_d[t0 + 128 * bb:t0 + 128 * (bb + 1),
                                      512 * ncol:512 * (ncol + 1)],
                            in_=osb[:])

    nc.compile()
    return nc


def _masks():
    p = np.arange(128)[:, None]
    f = np.arange(128)[None, :]
    triu = (f >= p).astype(np.float32)
    striu = (f > p).astype(np.float32)
    ones = np.ones((128, 128), np.float32)
    zeros = np.zeros((128, 128), np.float32)
    return {
        "ident": np.eye(128, dtype=np.float32),
        "mt0": np.concatenate([triu, ones], axis=1),
        "ms0": np.concatenate([striu, ones], axis=1),
        "mz1": np.concatenate([zeros, triu], axis=1),
        "mtr": triu,
        "mst": striu,
    }


_NC_CACHE = {}


def get_nc(T=2048):
    if T not in _NC_CACHE:
        _NC_CACHE[T] = build(T)
    return _NC_CACHE[T]


def kernel(x, W_q, W_k, W_v, W_o):
    T = x.shape[1]
    nc = get_nc(T)
    masks = _masks()
    in_maps = []
    for c in range(8):
        b, g = c // 4, c % 4
        im = {
            "x": np.ascontiguousarray(x[b]).astype(np.float32),
            "wq": np.ascontiguousarray(W_q[:, DL * g:DL * (g + 1)]).astype(np.float32),
            "wk": np.ascontiguousarray(W_k[:, DL * g:DL * (g + 1)]).astype(np.float32),
            "wv": np.ascontiguousarray(W_v[:, DL * g:DL * (g + 1)]).astype(np.float32),
            "wo": np.ascontiguousarray(W_o[DL * g:DL * (g + 1), :]).astype(np.float32),
        }
        im.update(masks)
        in_maps.append(im)
    res = run_bass_kernel_spmd(nc, in_maps, list(range(8)))
    global _last_res
    _last_res = res
    out = np.zeros((2, T, D), np.float32)
    for c in range(8):
        out[c // 4] += res.results[c]["out"]
    return out



# revision 3
# speedup vs baseline: 1.6484x; 1.6484x over previous
"""Trainium2 Bass kernel for nn_HLALayer (higher-order linear attention).

Math: the reference scan
    k_C = k_t @ C;  G += k_t k_t^T C;  S += k_t k_t^T;  C += q_t v_t^T
    o_t = q_t @ (S C - G)
admits a chunked closed form (chunk L):
    o_chunk = Q @ D0 + tril(Q S0 Q^T + A tril(A)^T) @ V,   A = Q K^T
with per-chunk state updates
    dS = K^T K;  dC = Q^T V;  G1 = G0 + dS C0 + K^T(stril(K Q^T) V)
    S1 = S0+dS; C1 = C0+dC; D1 = S1 C1 - G1

v2: bf16 operands everywhere (PSUM accumulation stays f32) -> FWL weight
loads, halved DMA; host passes x pre-transposed so no PE transposes of x;
trimmed triangular matmuls (right-half-only N1/M1/AT1); head-pair packing
of the 64-wide matmuls onto disjoint PE row/col groups.

Sharding: 8 cores = (batch b in {0,1}) x (head-group g in {0..3}, 4 heads
each).  Each core projects x[b] with its weight column/row slices, runs the
chunked scan for its 4 heads, applies its W_o row-slice -> partial [T, D]
bf16 output; the host sums the 4 partials per batch in f32.
"""

import numpy as np
import sys

sys.path.insert(0, "/opt/trn_rl_repo")

import ml_dtypes
import concourse.bacc as bacc
import concourse.mybir as mybir
from concourse.bass_utils import run_bass_kernel_spmd
from concourse.tile import TileContext

F32 = mybir.dt.float32
BF16 = mybir.dt.bfloat16
BF = ml_dtypes.bfloat16

D = 1024          # model dim
DL = 256          # per-core projection width (4 heads x 64)
DK = 64           # head dim
L = 256           # chunk length
NHL = 4           # local heads per core


def build(T=2048):
    NCH = T // L
    nc = bacc.Bacc("TRN2", target_bir_lowering=False)

    xt_in = nc.declare_dram_parameter("xt", [D, T], BF16, isOutput=False)
    wq_in = nc.declare_dram_parameter("wq", [D, DL], BF16, isOutput=False)
    wk_in = nc.declare_dram_parameter("wk", [D, DL], BF16, isOutput=False)
    wv_in = nc.declare_dram_parameter("wv", [D, DL], BF16, isOutput=False)
    wo_in = nc.declare_dram_parameter("wo", [DL, D], BF16, isOutput=False)
    id_in = nc.declare_dram_parameter("ident", [128, 128], BF16, isOutput=False)
    mtr_in = nc.declare_dram_parameter("mtr", [128, 128], F32, isOutput=False)  # triu
    mst_in = nc.declare_dram_parameter("mst", [128, 128], F32, isOutput=False)  # striu
    mt0_in = nc.declare_dram_parameter("mt0", [128, 256], F32, isOutput=False)  # [triu|1]
    out_d = nc.declare_dram_parameter("out", [T, D], BF16, isOutput=True)

    ncp = 0  # copy-engine round robin counter

    with TileContext(nc) as tc:
        with tc.tile_pool(name="const", bufs=1) as cpool, \
             tc.tile_pool(name="work", bufs=2) as work, \
             tc.tile_pool(name="spool", bufs=2) as spool, \
             tc.tile_pool(name="pp", bufs=2, space="PSUM") as pps:

            def cp(out_ap, in_ap):
                """plain copy/cast, alternating DVE / ACT to balance load"""
                nonlocal ncp
                ncp += 1
                if ncp % 2 == 0:
                    nc.vector.tensor_copy(out_ap, in_ap)
                else:
                    nc.scalar.copy(out_ap, in_ap)

            # ---- constants / weights (gpsimd=SWDGE queue, keeps HWDGE free) ----
            ident = cpool.tile([128, 128], BF16)
            nc.gpsimd.dma_start(out=ident[:], in_=id_in[:])
            mtr = cpool.tile([128, 128], F32)
            nc.gpsimd.dma_start(out=mtr[:], in_=mtr_in[:])
            mst = cpool.tile([128, 128], F32)
            nc.gpsimd.dma_start(out=mst[:], in_=mst_in[:])
            mt0 = cpool.tile([128, 256], F32)
            nc.gpsimd.dma_start(out=mt0[:], in_=mt0_in[:])

            wq_sb, wk_sb, wv_sb = [], [], []
            for j in range(8):
                wqt = cpool.tile([128, DL], BF16, name=f"wq{j}")
                nc.gpsimd.dma_start(out=wqt[:], in_=wq_in[128 * j:128 * (j + 1), :])
                wq_sb.append(wqt)
                wkt = cpool.tile([128, DL], BF16, name=f"wk{j}")
                nc.gpsimd.dma_start(out=wkt[:], in_=wk_in[128 * j:128 * (j + 1), :])
                wk_sb.append(wkt)
                wvt = cpool.tile([128, DL], BF16, name=f"wv{j}")
                nc.gpsimd.dma_start(out=wvt[:], in_=wv_in[128 * j:128 * (j + 1), :])
                wv_sb.append(wvt)
            wo_sb = []
            for m in range(2):
                wot = cpool.tile([128, D], BF16, name=f"wo{m}")
                nc.gpsimd.dma_start(out=wot[:], in_=wo_in[128 * m:128 * (m + 1), :])
                wo_sb.append(wot)

            # per-pair states (h0 at partitions 0:64, h1 at 64:128), bf16
            S = [None] * 2
            C = [None] * 2
            G = [None] * 2
            Dst = [None] * 2

            for c in range(NCH):
                t0 = L * c
                # ---------- phase 1: load x^T tiles, project ----------
                xt = []
                for j in range(8):
                    xtj = work.tile([128, L], BF16, tag=f"xt{j}", bufs=2,
                                    name=f"xt{j}_{c}")
                    nc.sync.dma_start(out=xtj[:],
                                      in_=xt_in[128 * j:128 * (j + 1), t0:t0 + L])
                    xt.append(xtj)

                qt, kt = [], []
                for m in range(2):
                    psq = pps.tile([128, L], F32, tag="pb", bufs=5, name=f"psq{m}_{c}")
                    for j in range(8):
                        nc.tensor.matmul(psq[:], wq_sb[j][:, 128 * m:128 * (m + 1)],
                                         xt[j][:], start=(j == 0), stop=(j == 7))
                    qtm = work.tile([128, L], BF16, tag=f"qt{m}", bufs=2,
                                    name=f"qt{m}_{c}")
                    cp(qtm[:], psq[:])
                    qt.append(qtm)
                    psk = pps.tile([128, L], F32, tag="pb", bufs=5, name=f"psk{m}_{c}")
                    for j in range(8):
                        nc.tensor.matmul(psk[:], wk_sb[j][:, 128 * m:128 * (m + 1)],
                                         xt[j][:], start=(j == 0), stop=(j == 7))
                    ktm = work.tile([128, L], BF16, tag=f"kt{m}", bufs=2,
                                    name=f"kt{m}_{c}")
                    cp(ktm[:], psk[:])
                    kt.append(ktm)
                vt = []
                for bb in range(2):
                    psv = pps.tile([128, DL], F32, tag="pb", bufs=5, name=f"psv{bb}_{c}")
                    for j in range(8):
                        nc.tensor.matmul(psv[:], xt[j][:, 128 * bb:128 * (bb + 1)],
                                         wv_sb[j][:], start=(j == 0), stop=(j == 7))
                    vtb = work.tile([128, DL], BF16, tag=f"vt{bb}", bufs=2,
                                    name=f"vt{bb}_{c}")
                    cp(vtb[:], psv[:])
                    vt.append(vtb)

                # per-chunk output tiles (oT layout [dv, t]; m=0: heads 0,1)
                ot = [work.tile([128, L], BF16, tag=f"ot{m}", bufs=2,
                                name=f"ot{m}_{c}") for m in range(2)]

                # ---------- phase 2: chunked scan, head pairs ----------
                for p in range(2):
                    heads = (2 * p, 2 * p + 1)
                    QT, KT, Vbs, idb = {}, {}, {}, {}
                    for h in heads:
                        m, po = h >> 1, 64 * (h & 1)
                        QT[h] = qt[m][po:po + 64, :]
                        KT[h] = kt[m][po:po + 64, :]
                        Vbs[h] = [vt[bb][:, 64 * h:64 * h + 64] for bb in range(2)]
                        idb[h] = ident[po:po + 64, po:po + 64]

                    # NM products: psNM = [N0 (256) | N1 right (128)], psM likewise
                    psNM, psM = {}, {}
                    for h in heads:
                        po = 64 * (h & 1)
                        psNM[h] = pps.tile([128, 384], F32, tag="pb", bufs=5,
                                           name=f"psNM_{c}_{h}")
                        nc.tensor.matmul(psNM[h][:, 0:256], KT[h][:, 0:128], QT[h],
                                         start=True, stop=True)
                        nc.tensor.matmul(psNM[h][:, 256:384], KT[h][:, 128:256],
                                         QT[h][:, 128:256], start=True, stop=True)
                    for h in heads:
                        psM[h] = pps.tile([128, 384], F32, tag="pb", bufs=5,
                                          name=f"psM_{c}_{h}")
                        nc.tensor.matmul(psM[h][:, 0:256], QT[h][:, 0:128], KT[h],
                                         start=True, stop=True)
                        nc.tensor.matmul(psM[h][:, 256:384], QT[h][:, 128:256],
                                         KT[h][:, 128:256], start=True, stop=True)

                    # natural q/k via PE transpose (dk 64 -> partitions)
                    qn, kn = {}, {}
                    for h in heads:
                        po = 64 * (h & 1)
                        qn[h], kn[h] = [], []
                        for bb in range(2):
                            psq_t = pps.tile([128, 64], BF16, tag="ps", bufs=3,
                                             name=f"psqn{bb}_{c}_{h}")
                            nc.tensor.transpose(psq_t[:],
                                                QT[h][:, 128 * bb:128 * (bb + 1)],
                                                idb[h])
                            qnb = work.tile([128, 64], BF16, tag=f"qn{bb}{h}", bufs=2,
                                            name=f"qn{bb}_{c}_{h}")
                            cp(qnb[:], psq_t[:])
                            qn[h].append(qnb)
                            psk_t = pps.tile([128, 64], BF16, tag="ps", bufs=3,
                                             name=f"pskn{bb}_{c}_{h}")
                            nc.tensor.transpose(psk_t[:],
                                                KT[h][:, 128 * bb:128 * (bb + 1)],
                                                idb[h])
                            knb = work.tile([128, 64], BF16, tag=f"kn{bb}{h}", bufs=2,
                                            name=f"kn{bb}_{c}_{h}")
                            cp(knb[:], psk_t[:])
                            kn[h].append(knb)

                    # masks / casts of N and M
                    n0sb, n1sbR, triuN0L, triuN1, smM0L, m0R, smM1 = (
                        {}, {}, {}, {}, {}, {}, {})
                    for h in heads:
                        n0sb[h] = work.tile([128, 256], BF16, tag=f"n0sb{h}", bufs=2,
                                            name=f"n0sb_{c}_{h}")
                        cp(n0sb[h][:], psNM[h][:, 0:256])
                        n1sbR[h] = work.tile([128, 128], BF16, tag=f"n1sbR{h}", bufs=2,
                                             name=f"n1sbR_{c}_{h}")
                        cp(n1sbR[h][:], psNM[h][:, 256:384])
                        triuN0L[h] = work.tile([128, 128], BF16, tag=f"tN0{h}", bufs=2,
                                               name=f"tN0_{c}_{h}")
                        nc.vector.tensor_mul(triuN0L[h][:], psNM[h][:, 0:128], mtr[:])
                        triuN1[h] = work.tile([128, 128], BF16, tag=f"tN1{h}", bufs=2,
                                              name=f"tN1_{c}_{h}")
                        nc.vector.tensor_mul(triuN1[h][:], psNM[h][:, 256:384], mtr[:])
                        smM0L[h] = work.tile([128, 128], BF16, tag=f"sM0{h}", bufs=2,
                                             name=f"sM0_{c}_{h}")
                        nc.vector.tensor_mul(smM0L[h][:], psM[h][:, 0:128], mst[:])
                        m0R[h] = work.tile([128, 128], BF16, tag=f"m0R{h}", bufs=2,
                                           name=f"m0R_{c}_{h}")
                        cp(m0R[h][:], psM[h][:, 128:256])
                        smM1[h] = work.tile([128, 128], BF16, tag=f"sM1{h}", bufs=2,
                                            name=f"sM1_{c}_{h}")
                        nc.vector.tensor_mul(smM1[h][:], psM[h][:, 256:384], mst[:])

                    # QST = S0 @ QT per head, pair-packed [128, 256]
                    qstsb = None
                    if c > 0:
                        psQST = pps.tile([128, L], F32, tag="ps", bufs=3,
                                         name=f"psQST_{c}_{p}")
                        for h in heads:
                            po = 64 * (h & 1)
                            nc.tensor.matmul(psQST[po:po + 64, :],
                                             S[p][po:po + 64, :], QT[h],
                                             start=True, stop=True)
                        qstsb = work.tile([128, L], BF16, tag=f"qst{p}", bufs=2,
                                          name=f"qst_{c}_{p}")
                        cp(qstsb[:], psQST[:])

                    # AT = [AT0 (r 0:128, t 0:256) | AT1 right (r 128:256, t 128:256)]
                    psAT, at0, at1R = {}, {}, {}
                    for h in heads:
                        po = 64 * (h & 1)
                        psAT[h] = pps.tile([128, 384], F32, tag="pb", bufs=5,
                                           name=f"psAT_{c}_{h}")
                        nc.tensor.matmul(psAT[h][:, 0:256], triuN0L[h][:], n0sb[h][:],
                                         start=True, stop=(c == 0))
                        if c > 0:
                            nc.tensor.matmul(psAT[h][:, 0:256],
                                             qstsb[po:po + 64, 0:128], QT[h],
                                             start=False, stop=True)
                        nc.tensor.matmul(psAT[h][:, 256:384], n0sb[h][:, 128:256],
                                         n0sb[h][:, 128:256], start=True, stop=False)
                        nc.tensor.matmul(psAT[h][:, 256:384], triuN1[h][:],
                                         n1sbR[h][:], start=False, stop=(c == 0))
                        if c > 0:
                            nc.tensor.matmul(psAT[h][:, 256:384],
                                             qstsb[po:po + 64, 128:256],
                                             QT[h][:, 128:256],
                                             start=False, stop=True)
                    for h in heads:
                        at0[h] = work.tile([128, 256], BF16, tag=f"at0{h}", bufs=2,
                                           name=f"at0_{c}_{h}")
                        nc.vector.tensor_mul(at0[h][:], psAT[h][:, 0:256], mt0[:])
                        at1R[h] = work.tile([128, 128], BF16, tag=f"at1{h}", bufs=2,
                                            name=f"at1_{c}_{h}")
                        nc.vector.tensor_mul(at1R[h][:], psAT[h][:, 256:384], mtr[:])

                    # oT = V^T tril(AT) + (D0 Q^T), pair-packed [128, 256]
                    psO = pps.tile([128, L], F32, tag="ps", bufs=3,
                                   name=f"psO_{c}_{p}")
                    for h in heads:
                        po = 64 * (h & 1)
                        nc.tensor.matmul(psO[po:po + 64, :], Vbs[h][0], at0[h][:],
                                         start=True, stop=False)
                        nc.tensor.matmul(psO[po:po + 64, 128:256], Vbs[h][1],
                                         at1R[h][:], start=False, stop=(c == 0))
                        if c > 0:
                            nc.tensor.matmul(psO[po:po + 64, :],
                                             Dst[p][po:po + 64, :], QT[h],
                                             start=False, stop=True)
                    cp(ot[p][:], psO[:])

                    # dS = K^T K, dC = Q^T V (pair-packed [128, 64])
                    psS = pps.tile([128, 64], F32, tag="ps", bufs=3,
                                   name=f"psS_{c}_{p}")
                    psC = pps.tile([128, 64], F32, tag="ps", bufs=3,
                                   name=f"psC_{c}_{p}")
                    for h in heads:
                        po = 64 * (h & 1)
                        nc.tensor.matmul(psS[po:po + 64, :], kn[h][0][:], kn[h][0][:],
                                         start=True, stop=False)
                        nc.tensor.matmul(psS[po:po + 64, :], kn[h][1][:], kn[h][1][:],
                                         start=False, stop=True)
                        nc.tensor.matmul(psC[po:po + 64, :], qn[h][0][:], Vbs[h][0],
                                         start=True, stop=False)
                        nc.tensor.matmul(psC[po:po + 64, :], qn[h][1][:], Vbs[h][1],
                                         start=False, stop=True)

                    # W2 = stril(KQ^T) V  per head: psW = [W2(s 0:128) | W2(s 128:256)]
                    wsb = {}
                    for h in heads:
                        psW = pps.tile([128, 128], F32, tag="ps", bufs=3,
                                       name=f"psW_{c}_{h}")
                        nc.tensor.matmul(psW[:, 0:64], smM0L[h][:], Vbs[h][0],
                                         start=True, stop=True)
                        nc.tensor.matmul(psW[:, 64:128], m0R[h][:], Vbs[h][0],
                                         start=True, stop=False)
                        nc.tensor.matmul(psW[:, 64:128], smM1[h][:], Vbs[h][1],
                                         start=False, stop=True)
                        wsb[h] = work.tile([128, 128], BF16, tag=f"wsb{h}", bufs=2,
                                           name=f"wsb_{c}_{h}")
                        cp(wsb[h][:], psW[:])

                    # Gamma = K^T W2 (+ dS C0), pair-packed
                    dssb = None
                    if c > 0:
                        dssb = work.tile([128, 64], BF16, tag=f"dssb{p}", bufs=2,
                                         name=f"dssb_{c}_{p}")
                        cp(dssb[:], psS[:])
                    psG = pps.tile([128, 64], F32, tag="ps", bufs=3,
                                   name=f"psG_{c}_{p}")
                    for h in heads:
                        po = 64 * (h & 1)
                        nc.tensor.matmul(psG[po:po + 64, :], kn[h][0][:],
                                         wsb[h][:, 0:64], start=True, stop=False)
                        nc.tensor.matmul(psG[po:po + 64, :], kn[h][1][:],
                                         wsb[h][:, 64:128], start=False, stop=(c == 0))
                        if c > 0:
                            nc.tensor.matmul(psG[po:po + 64, :], dssb[po:po + 64, :],
                                             C[p][po:po + 64, :],
                                             start=False, stop=True)

                    # new states (one DVE op per state for the pair)
                    Snew = spool.tile([128, 64], BF16, tag=f"S{p}", bufs=2,
                                      name=f"S{p}_{c}")
                    Cnew = spool.tile([128, 64], BF16, tag=f"C{p}", bufs=2,
                                      name=f"C{p}_{c}")
                    Gnew = spool.tile([128, 64], BF16, tag=f"G{p}", bufs=2,
                                      name=f"G{p}_{c}")
                    if c > 0:
                        nc.vector.tensor_add(Snew[:], S[p][:], psS[:])
                        nc.vector.tensor_add(Cnew[:], C[p][:], psC[:])
                        nc.vector.tensor_add(Gnew[:], G[p][:], psG[:])
                    else:
                        nc.vector.tensor_copy(Snew[:], psS[:])
                        nc.vector.tensor_copy(Cnew[:], psC[:])
                        nc.vector.tensor_copy(Gnew[:], psG[:])
                    psD = pps.tile([128, 64], F32, tag="ps", bufs=3,
                                   name=f"psD_{c}_{p}")
                    for h in heads:
                        po = 64 * (h & 1)
                        nc.tensor.matmul(psD[po:po + 64, :], Snew[po:po + 64, :],
                                         Cnew[po:po + 64, :], start=True, stop=True)
                    Dnew = spool.tile([128, 64], BF16, tag=f"D{p}", bufs=2,
                                      name=f"D{p}_{c}")
                    nc.vector.tensor_sub(Dnew[:], psD[:], Gnew[:])
                    S[p], C[p], G[p], Dst[p] = Snew, Cnew, Gnew, Dnew

                # ---------- phase 3: output projection for this chunk ----------
                for bb in range(2):
                    for ncol in range(2):
                        pso = pps.tile([128, 512], F32, tag="pb", bufs=5,
                                       name=f"pso{bb}{ncol}_{c}")
                        nc.tensor.matmul(pso[:], ot[0][:, 128 * bb:128 * (bb + 1)],
                                         wo_sb[0][:, 512 * ncol:512 * (ncol + 1)],
                                         start=True, stop=False)
                        nc.tensor.matmul(pso[:], ot[1][:, 128 * bb:128 * (bb + 1)],
                                         wo_sb[1][:, 512 * ncol:512 * (ncol + 1)],
                                         start=False, stop=True)
                        osb = work.tile([128, 512], BF16, tag="osb", bufs=3,
                                        name=f"osb{bb}{ncol}_{c}")
                        cp(osb[:], pso[:])
                        nc.scalar.dma_start(
                            out=out_d[t0 + 128 * bb:t0 + 128 * (bb + 1),
                                      512 * ncol:512 * (ncol + 1)],
                            in_=osb[:])

    nc.compile()
    return nc


def _masks():
    p = np.arange(128)[:, None]
    f = np.arange(128)[None, :]
    triu = (f >= p).astype(np.float32)
    striu = (f > p).astype(np.float32)
    ones = np.ones((128, 128), np.float32)
    return {
        "ident": np.eye(128, dtype=BF),
        "mtr": triu,
        "mst": striu,
        "mt0": np.concatenate([triu, ones], axis=1),
    }


_NC_CACHE = {}


def get_nc(T=2048):
    if T not in _NC_CACHE:
        _NC_CACHE[T] = build(T)
    return _NC_CACHE[T]


def kernel(x, W_q, W_k, W_v, W_o):
    T = x.shape[1]
    nc = get_nc(T)
    masks = _masks()
    xts = [np.ascontiguousarray(x[b].T).astype(BF) for b in range(2)]
    in_maps = []
    for cc in range(8):
        b, g = cc // 4, cc % 4
        im = {
            "xt": xts[b],
            "wq": np.ascontiguousarray(W_q[:, DL * g:DL * (g + 1)]).astype(BF),
            "wk": np.ascontiguousarray(W_k[:, DL * g:DL * (g + 1)]).astype(BF),
            "wv": np.ascontiguousarray(W_v[:, DL * g:DL * (g + 1)]).astype(BF),
            "wo": np.ascontiguousarray(W_o[DL * g:DL * (g + 1), :]).astype(BF),
        }
        im.update(masks)
        in_maps.append(im)
    res = run_bass_kernel_spmd(nc, in_maps, list(range(8)))
    global _last_res
    _last_res = res
    out = np.zeros((2, T, D), np.float32)
    for cc in range(8):
        out[cc // 4] += np.asarray(res.results[cc]["out"], dtype=np.float32)
    return out
